# revision 20
# baseline (speedup 1.0000x reference)
"""BankedLinear (MoE-style banked linear) Trainium2 Bass kernel.

Math: out[n] = sum_k bank_weights[n,k] * (tensor[n] @ W[sel[n,k]] + bias[sel[n,k]])
Shapes: tensor [8192,128] f32, bank_weights [8192,2] f32, bank_selections [8192,2] int,
        weights [64,128,128] f32, bias [64,128] f32 -> out [8192,128] f32.

Strategy (data parallel over tokens, bf16 compute, two sorted passes):
  - 8 cores x 1024 tokens, greedy-balanced so per-bank per-pass counts are
    nearly equal across cores (SPMD: one program, shared bank capacities).
  - Pass A handles every token's k=0 pair sorted by sel0; pass B handles k=1
    sorted by sel1. Each pass:
      x slots loaded straight into transposed SBUF layout via dma_transpose
      (bf16), then per-bank matmuls W_b^T @ xT accumulate y^T into psum
      ([128 out, token cols], bf16 => 1 cycle/row). The bias term is seeded
      into the same psum by a rank-64 matmul bias^T @ H (H = 0/1 one-hot of
      the slot's bank, host built).
  - psum -> SBUF bf16 copy, PE transpose back to row layout, and the
    psum->SBUF copy of the transpose applies the per-token bank_weight as a
    tensor_scalar multiply (free).
  - Pass A rows are stored contiguously to the DRAM output; pass B rows are
    dma_scatter_add-ed (SWDGE, descriptors pre-generated via prepare_only)
    into the same buffer at the A-slot of the same token. Pad slots compute
    exact zeros and are pointed at row 0 (add of 0) or dropped by the host.
  - Host unpermutes: out[token_of_A_slot] = out_big[slot].
"""

import numpy as np
import ml_dtypes

N, K, IN, OUT, NUM_BANKS = 8192, 2, 128, 128, 64
NCORES = 8
NLOC = N // NCORES  # tokens per core
P = 128
PSUM_FREE = 512  # fp32 columns per psum bank
BF16 = ml_dtypes.bfloat16


def _routing_plan(sel_all):
    """Balance tokens across cores so that per-core per-bank counts of sel0
    and sel1 are close to the global ideal. Returns (assign [NCORES, NLOC],
    caps0, caps1) with caps shared by all cores (SPMD program)."""
    sel_all = np.asarray(sel_all).astype(np.int64)
    g0 = np.bincount(sel_all[:, 0], minlength=NUM_BANKS)
    g1 = np.bincount(sel_all[:, 1], minlength=NUM_BANKS)
    ideal0 = (g0 + NCORES - 1) // NCORES
    ideal1 = (g1 + NCORES - 1) // NCORES
    c0 = np.zeros((NCORES, NUM_BANKS), dtype=np.int64)
    c1 = np.zeros((NCORES, NUM_BANKS), dtype=np.int64)
    fill = np.zeros(NCORES, dtype=np.int64)
    assign_lists = [[] for _ in range(NCORES)]
    for n in range(N):
        b0, b1 = int(sel_all[n, 0]), int(sel_all[n, 1])
        best, best_key = -1, None
        for c in range(NCORES):
            if fill[c] >= NLOC:
                continue
            over = max(0, c0[c, b0] + 1 - ideal0[b0]) + max(
                0, c1[c, b1] + 1 - ideal1[b1])
            key = (over, c0[c, b0] + c1[c, b1], fill[c])
            if best < 0 or key < best_key:
                best, best_key = c, key
        c0[best, b0] += 1
        c1[best, b1] += 1
        fill[best] += 1
        assign_lists[best].append(n)
    assign = np.array(assign_lists, dtype=np.int64)

    caps0 = c0.max(axis=0).astype(np.int64)
    caps1 = c1.max(axis=0).astype(np.int64)
    # pad total slot counts to a multiple of 128 by growing the last bank
    caps0[NUM_BANKS - 1] += (-int(caps0.sum())) % P
    caps1[NUM_BANKS - 1] += (-int(caps1.sum())) % P
    return assign, caps0, caps1


def _offsets(caps):
    return np.concatenate([[0], np.cumsum(caps)[:-1]]).astype(np.int64)


def _segments(caps, offs):
    """Shared matmul segment list: (psum_tile, col0_in_tile, width, bank),
    bank ranges split at psum-tile (512 col) boundaries."""
    segs = []
    for b in range(NUM_BANKS):
        if caps[b] == 0:
            continue
        s0, s1 = int(offs[b]), int(offs[b] + caps[b])
        while s0 < s1:
            ti = s0 // PSUM_FREE
            e = min(s1, (ti + 1) * PSUM_FREE)
            segs.append((ti, s0 - ti * PSUM_FREE, e - s0, b))
            s0 = e
    return segs


def _wrap_idx(flat_idx):
    """Wrap a flat int16 index list into the [128, n//16] SWDGE layout:
    index i lives at [i % 16, i // 16], replicated across the 8 Q7 groups."""
    n = flat_idx.shape[0]
    assert n % 16 == 0
    w = flat_idx.reshape(n // 16, 16).T.astype(np.int16)
    return np.tile(w, (8, 1))


def _build_program(C0, C1, segsA, segsB):
    import concourse.bacc as bacc
    import concourse.tile as tile
    from concourse import mybir, library_config
    from concourse.masks import make_identity
    from concourse.tile import add_dep_helper

    f32 = mybir.dt.float32
    bf16 = mybir.dt.bfloat16
    i16 = mybir.dt.int16

    nbA, nbB = C0 // P, C1 // P  # 128-row blocks per pass
    tilesA = [min(PSUM_FREE, C0 - t) for t in range(0, C0, PSUM_FREE)]
    tilesB = [min(PSUM_FREE, C1 - t) for t in range(0, C1, PSUM_FREE)]
    assert len(tilesA) <= 3 and len(tilesB) <= 3, (C0, C1)

    nc = bacc.Bacc(None, target_bir_lowering=False, debug=False)

    xA_d = nc.declare_dram_parameter("xa", [C0, IN], bf16, isOutput=False)
    xB_d = nc.declare_dram_parameter("xb", [C1, IN], bf16, isOutput=False)
    w_d = nc.declare_dram_parameter("wts", [IN, NUM_BANKS * OUT], bf16,
                                    isOutput=False)
    # merged small loads: [bias | H0 | H1] on 64 partitions,
    # [sidx | bw-bitcast] on 128 partitions (fewer DMAs = shorter HWDGE chain)
    m64_d = nc.declare_dram_parameter("m64", [NUM_BANKS, OUT + C0 + C1], bf16,
                                      isOutput=False)
    m128_w = (C0 + C1) // 16 + 2 * ((C0 + C1) // P)
    m128_d = nc.declare_dram_parameter("m128", [P, m128_w], i16,
                                       isOutput=False)
    # extra dump rows: every pad slot of pass B scatter-adds into its own
    # private row (concurrent adds to a shared row race on real SWDGE)
    npad = C1 - NLOC
    out_d = nc.declare_dram_parameter("out", [C0 + npad, OUT], f32,
                                      isOutput=True)

    nblk_out = (C0 + (C1 - NLOC)) // P

    with tile.TileContext(nc) as tc:
        with (
            tc.tile_pool(name="const", bufs=1) as cpool,
            tc.tile_pool(name="big", bufs=1) as bigpool,
            tc.tile_pool(name="psum_a", bufs=1, space="PSUM") as psum_a,
            tc.tile_pool(name="psum_b", bufs=1, space="PSUM") as psum_b,
            tc.tile_pool(name="psum_t", bufs=2, space="PSUM") as psum_t,
        ):
            # ACT ring carries the whole HWDGE chain (one engine: the
            # scheduler's global DMA order costs no cross-engine waits)
            m64_sb = cpool.tile([NUM_BANKS, OUT + C0 + C1], bf16)
            nc.scalar.dma_start(out=m64_sb[:], in_=m64_d.ap())
            bias_sb = m64_sb[:, :OUT]
            h_sb = m64_sb[:, OUT:]
            m128_sb = cpool.tile([P, m128_w], i16)
            nc.scalar.dma_start(out=m128_sb[:], in_=m128_d.ap())
            sidx_sb = m128_sb[:, :(C0 + C1) // 16]
            bw_sb = m128_sb[:, (C0 + C1) // 16:].bitcast(f32)
            xTA = bigpool.tile([P, C0], bf16, tag="xTA")
            nc.scalar.dma_start_transpose(xTA[:], xA_d.ap())
            xTB = bigpool.tile([P, C1], bf16, tag="xTB")
            nc.scalar.dma_start_transpose(xTB[:], xB_d.ap())
            # prime the ACT Copy LUT before the first real activation op
            warm = cpool.tile([P, 1], f32)
            nc.vector.memset(warm[:], 0.0)
            nc.scalar.activation(warm[:], warm[:],
                                 mybir.ActivationFunctionType.Copy)

            # Pool/SWDGE ring: weights (escapes the HWDGE chain), zeroed
            # output buffer (both passes scatter-add into it, no ordering)
            ident = cpool.tile([P, P], bf16)
            make_identity(nc, ident[:])
            libload = nc.gpsimd.load_library(library_config.mlp)
            wT = bigpool.tile([P, NUM_BANKS * OUT], bf16, tag="wT")
            half = (NUM_BANKS // 2) * OUT
            nc.gpsimd.dma_start(out=wT[:, :half], in_=w_d[:, :half])
            nc.gpsimd.dma_start(out=wT[:, half:], in_=w_d[:, half:])
            ztile = bigpool.tile([P, nblk_out, OUT], f32, tag="ztile")
            nc.vector.memset(ztile[:], 0.0)
            zstore = nc.gpsimd.dma_start(
                out=out_d.ap().rearrange("(t p) o -> p t o", p=P),
                in_=ztile[:])

            # psum tiles per pass
            psA = [psum_a.tile([P, w], f32, tag=f"a{i}", name=f"psa{i}")
                   for i, w in enumerate(tilesA)]
            psB = [psum_b.tile([P, w], f32, tag=f"b{i}", name=f"psb{i}")
                   for i, w in enumerate(tilesB)]

            def run_pass(ps, tiles, segs, h_off, ti):
                """Per-segment bias seed + weight matmul for psum tile ti.
                Seed: psum[o, i] = sum_b bias[b, o] * H[b, i], then the
                bank's W matmul accumulates on top of it."""
                xT = xTA if h_off == 0 else xTB
                for (t, c0, cw, b) in segs:
                    if t != ti:
                        continue
                    g0 = t * PSUM_FREE + c0
                    nc.tensor.matmul(
                        out=ps[ti][:, c0:c0 + cw],
                        lhsT=bias_sb[:],
                        rhs=h_sb[:, h_off + g0:h_off + g0 + cw],
                        start=True, stop=False,
                    )
                    nc.tensor.matmul(
                        out=ps[ti][:, c0:c0 + cw],
                        lhsT=wT[:, b * OUT:(b + 1) * OUT],
                        rhs=xT[:, g0:g0 + cw],
                        start=False, stop=True,
                    )

            # W1-era tiles first, then W2-era (PE SEQ is in order)
            tile_order = []
            for i in range(max(len(tilesA), len(tilesB))):
                if i < len(tilesA):
                    tile_order.append(("A", i))
                if i < len(tilesB):
                    tile_order.append(("B", i))
            for (side, i) in tile_order:
                if side == "A":
                    run_pass(psA, tilesA, segsA, 0, i)
                else:
                    run_pass(psB, tilesB, segsB, C0, i)

            # psum^T -> SBUF bf16, PE transpose back, scaled copy to rows
            aT = bigpool.tile([P, C0], bf16, tag="aT")
            bT = bigpool.tile([P, C1], bf16, tag="bT")
            o0 = bigpool.tile([P, nbA, OUT], f32, tag="o0")
            o1 = bigpool.tile([P, nbB, OUT], f32, tag="o1")

            eng_i = 0

            def copy_eng():
                nonlocal eng_i
                eng_i += 1
                return nc.vector if eng_i % 2 == 0 else nc.scalar

            def finish_tile(side, ti):
                tw = (tilesA if side == "A" else tilesB)[ti]
                ps = (psA if side == "A" else psB)[ti]
                yT = aT if side == "A" else bT
                orows = o0 if side == "A" else o1
                bw_off = 0 if side == "A" else nbA
                col0 = ti * PSUM_FREE
                # big psum -> SBUF bf16 copy
                e = copy_eng()
                if e is nc.vector:
                    nc.vector.tensor_copy(yT[:, col0:col0 + tw], ps[:, :])
                else:
                    nc.scalar.copy(yT[:, col0:col0 + tw], ps[:, :])
                # per 128-col block: PE transpose back + scaled copy
                for j0 in range(0, tw, P):
                    j = (col0 + j0) // P
                    pt = psum_t.tile([P, P], bf16, tag="pt")
                    nc.tensor.transpose(out=pt[:], in_=yT[:, col0 + j0:
                                                          col0 + j0 + P],
                                        identity=ident[:])
                    e = copy_eng()
                    if e is nc.vector:
                        nc.vector.tensor_scalar_mul(
                            orows[:, j, :], pt[:], bw_sb[:, bw_off + j:
                                                         bw_off + j + 1])
                    else:
                        nc.scalar.activation(
                            orows[:, j, :], pt[:],
                            mybir.ActivationFunctionType.Copy,
                            scale=bw_sb[:, bw_off + j:bw_off + j + 1])

            for (side, i) in tile_order:
                finish_tile(side, i)

            # both passes scatter-add into the zeroed output buffer;
            # descriptors prepped early, trigger fires once data is ready
            for side, orows, nb, idx_off in (
                ("a", o0, nbA, 0), ("b", o1, nbB, nbA)):
                dma_sem = nc.alloc_semaphore(f"swdge_scat_{side}")
                prep = nc.gpsimd.dma_scatter_add(
                    out_d.ap(),
                    orows[:, :, :],
                    sidx_sb[:, idx_off * 8:(idx_off + nb) * 8],
                    nb * P, nb * P, OUT,
                    prepare_only=True, sem=dma_sem,
                )
                add_dep_helper(prep.ins, libload.ins, sync=False,
                               reason="scatter needs mlp gpsimd library")
                trig = nc.gpsimd.trigger_dma(count=None)
                add_dep_helper(trig.ins, zstore.ins, sync=True,
                               reason="scatter adds into zeroed buffer")

    return nc


def _make_in_maps(tensor, bank_weights, bank_selections, weights, bias,
                  assign, caps0, caps1):
    tensor = np.asarray(tensor, dtype=np.float32)
    bank_weights = np.asarray(bank_weights, dtype=np.float32)
    sel_all = np.asarray(bank_selections).astype(np.int64)
    offs0, offs1 = _offsets(caps0), _offsets(caps1)
    C0, C1 = int(caps0.sum()), int(caps1.sum())

    wT = np.ascontiguousarray(
        np.asarray(weights, dtype=np.float32).transpose(1, 0, 2)
        .reshape(IN, NUM_BANKS * OUT)).astype(BF16)
    bias_bf = np.asarray(bias, dtype=np.float32).astype(BF16)

    in_maps = []
    slotA_all = []
    for c in range(NCORES):
        toks = assign[c]
        sel = sel_all[toks]          # [NLOC, K]
        bw = bank_weights[toks]      # [NLOC, K]
        x_bf = tensor[toks].astype(BF16)   # [NLOC, IN]

        # slot maps: slot -> local token (or -1 pad), per pass
        slotA = np.full(C0, -1, dtype=np.int64)
        slotB = np.full(C1, -1, dtype=np.int64)
        a_of_tok = np.zeros(NLOC, dtype=np.int64)
        fill0, fill1 = offs0.copy(), offs1.copy()
        for i in range(NLOC):
            b0, b1 = sel[i, 0], sel[i, 1]
            s = fill0[b0]; fill0[b0] += 1
            slotA[s] = i; a_of_tok[i] = s
            s = fill1[b1]; fill1[b1] += 1
            slotB[s] = i

        validA, validB = slotA >= 0, slotB >= 0
        xa = np.zeros((C0, IN), dtype=BF16)
        xa[validA] = x_bf[slotA[validA]]
        xb = np.zeros((C1, IN), dtype=BF16)
        xb[validB] = x_bf[slotB[validB]]

        h01 = np.zeros((NUM_BANKS, C0 + C1), dtype=BF16)
        h01[sel[slotA[validA], 0], np.nonzero(validA)[0]] = 1
        h01[sel[slotB[validB], 1], C0 + np.nonzero(validB)[0]] = 1

        bwab = np.zeros(C0 + C1, dtype=np.float32)
        bwab[:C0][validA] = bw[slotA[validA], 0]
        bwab[C0:][validB] = bw[slotB[validB], 1]
        # column-major fold: slot j*128+p at [p, j]
        bwab = np.concatenate([
            bwab[:C0].reshape(C0 // P, P).T,
            bwab[C0:].reshape(C1 // P, P).T], axis=1)

        # scatter indices. Pass A: identity (row = own slot, pads add zero to
        # their own row). Pass B: A slot of the same token; each pad slot adds
        # its zero row into a private dump row past C0 (shared rows race).
        sidxB = np.zeros(C1, dtype=np.int64)
        sidxB[validB] = a_of_tok[slotB[validB]]
        sidxB[~validB] = C0 + np.arange(int((~validB).sum()))
        sidx_w = np.concatenate(
            [_wrap_idx(np.arange(C0, dtype=np.int64)), _wrap_idx(sidxB)],
            axis=1)

        m64 = np.concatenate([bias_bf, h01], axis=1)
        m128 = np.concatenate(
            [sidx_w, np.ascontiguousarray(bwab).view(np.int16)], axis=1)
        in_maps.append({
            "xa": xa,
            "xb": xb,
            "wts": wT,
            "m64": np.ascontiguousarray(m64),
            "m128": np.ascontiguousarray(m128),
            "out": np.zeros((C0 + (C1 - NLOC), OUT), dtype=np.float32),
        })
        slotA_all.append(slotA)
    return in_maps, slotA_all


def kernel(tensor, bank_weights, bank_selections, weights, bias):
    tensor = np.asarray(tensor)
    bank_weights = np.asarray(bank_weights)
    bank_selections = np.asarray(bank_selections)
    weights = np.asarray(weights)
    bias = np.asarray(bias)

    assign, caps0, caps1 = _routing_plan(bank_selections)
    offs0, offs1 = _offsets(caps0), _offsets(caps1)
    C0, C1 = int(caps0.sum()), int(caps1.sum())
    segsA = _segments(caps0, offs0)
    segsB = _segments(caps1, offs1)
    nc = _build_program(C0, C1, segsA, segsB)
    in_maps, slotA_all = _make_in_maps(
        tensor, bank_weights, bank_selections, weights, bias,
        assign, caps0, caps1)

    nc.finalize()
    from concourse.bass_utils import run_bass_kernel_spmd
    try:
        res = run_bass_kernel_spmd(nc, in_maps, list(range(NCORES)))
    except Exception:
        # one retry: a previous crashed session can leave the accelerator in
        # a transient bad state that clears on the next dispatch
        import time
        time.sleep(2.0)
        res = run_bass_kernel_spmd(nc, in_maps, list(range(NCORES)))

    C0 = int(caps0.sum())
    out = np.empty((N, OUT), dtype=np.float32)
    for c in range(NCORES):
        ob = np.asarray(res.results[c]["out"])[:C0]
        slotA = slotA_all[c]
        m = slotA >= 0
        out[assign[c][slotA[m]]] = ob[m]
    return out
